# revision 1
# baseline (speedup 1.0000x reference)
"""Trainium2 Bass kernel for nn_ExpertLinear (dense MoE routing).

y[t, o] = sum_e weights[t, e] * (x[t, :] @ W[e] + b[e])

Strategy
--------
Data-parallel over the batch across 8 NeuronCores (2048 tokens per core);
W and b are replicated.  Per core:

  * All matmuls run in fp16 (1 cycle/row on the PE vs 4 for fp32) with fp32
    PSUM accumulation.  fp16's 11-bit significand keeps the final relative
    error ~3e-4, far better than bf16.
  * W is streamed in, cast to fp16 once, and kept fully resident in SBUF
    (16 MB).  x is cast to fp16 and transposed on the PE (x^T tiles are the
    stationary matmul operand, 4 MB resident); all transposes are scheduled
    inside the W-load window, where the PE has idle slack.
  * Token tiles are processed in blocks with the expert loop outside the
    tile loop, so each expert's W k-tiles unlock a full block of chain work
    (hides the W stream behind compute).  The first block is 4 tiles wide
    to match the W-DMA rate; later blocks are 2 wide.
  * For each (token-tile, expert): an 8-step PSUM accumulation chain
    computes x @ W[e] for 128 tokens x 512 outputs; the routing weight is
    applied output-side with a fused DVE scalar_tensor_tensor
    (y0 += w[:, e] * psum) using the per-partition scalar broadcast
    (tokens live on partitions in the output layout).
  * The bias term sum_e w[t,e] b[e,o] is a K=8 matmul (w^T tile [8 x 128]
    against b16 [8 x 1024]) that initializes the accumulator.
"""

import numpy as np

import concourse.bacc as bacc
import concourse.bass as bass
import concourse.mybir as mybir
import concourse.tile as tile
from concourse.bass_utils import run_bass_kernel_spmd
from concourse.masks import make_identity

EXPERTS = 8
IN_DIM = 1024
OUT_DIM = 1024
BATCH = 16384
N_CORES = 8

P = 128                 # partitions
T = BATCH // N_CORES    # tokens per core (2048)
TT = T // P             # token tiles per core (16)
KI = IN_DIM // P        # contraction tiles per expert (8)
NK = EXPERTS * KI       # total contraction tiles (64)
OC = 512                # psum free-dim chunk (one fp32 PSUM bank)

f32 = mybir.dt.float32
f16 = mybir.dt.float16


def _emit(tc, y, x, w, Wf, bf, T=T):
    nc = tc.nc
    TT = T // P
    BLK0 = min(4, TT)         # token tiles in the first block
    # Later blocks are 2 tiles: W is fully resident by then, and smaller
    # blocks keep fewer y-accumulators alive.
    blocks = [list(range(BLK0))]
    nxt = BLK0
    while nxt < TT:
        sz = min(4, TT - nxt)
        blocks.append(list(range(nxt, nxt + sz)))
        nxt += sz

    with (
        tc.tile_pool(name="big", bufs=1) as big,
        tc.tile_pool(name="stage", bufs=2) as stage,
        tc.tile_pool(name="yacc", bufs=BLK0) as yaccp,
        tc.tile_pool(name="ps", bufs=8, space="PSUM") as psp,
    ):
        ident = big.tile([P, P], f32)
        make_identity(nc, ident)
        ident16 = big.tile([P, P], f16)
        nc.vector.tensor_copy(ident16[:], ident[:])

        # Routing weights, token-on-partition layout: w_sb[p, t, e] = w[t*128+p, e].
        # One small DMA per token tile (descriptor-bound), on the SWDGE
        # queue so it delays neither the W stream (sync) nor x loads (scalar).
        w_sb = big.tile([P, TT, EXPERTS], f32)
        for t in range(TT):
            nc.gpsimd.dma_start(w_sb[:, t, :], w[t * P:(t + 1) * P, :])

        # Bias in fp16, experts on partitions (casting DMA on SWDGE).
        b16 = big.tile([EXPERTS, OUT_DIM], f16)
        nc.gpsimd.dma_start(b16[:], bf[:])

        # w^T tiles for the bias matmuls: wT16[e, t*128+j] = w[t*128+j, e]
        wT16 = big.tile([EXPERTS, TT * P], f16)

        W16 = big.tile([P, NK, OUT_DIM], f16)   # W [(e,i), o], fp16 resident
        xT16 = big.tile([P, KI, T], f16)        # x^T [i, tok], fp16 resident

        def prep_x_tile(t):
            """Load one x tile (scalar HWDGE queue), cast to fp16 on DVE,
            PE-transpose in fp16 into the resident x^T."""
            xs = stage.tile([P, IN_DIM], f32, tag="xstg", bufs=1,
                            name=f"xs_{t}")
            nc.scalar.dma_start(xs[:], x[t * P:(t + 1) * P, :])
            x16s = stage.tile([P, IN_DIM], f16, tag="x16s", bufs=2,
                              name=f"x16s_{t}")
            nc.vector.tensor_copy(x16s[:], xs[:])
            # All 8 transposed k-slices land in ONE single-bank fp16 PSUM
            # tile, drained by a single ACT copy — avoids PSUM slot churn
            # against the matmul chains.
            px = psp.tile([P, KI * P], f16, tag="ps", name=f"px_{t}")
            for j in range(KI):
                q, s = divmod(j, 2)
                nc.tensor.transpose(px[:, j * P:(j + 1) * P],
                                    x16s[:, 2 * P * q + s::2][:, :P],
                                    ident16[:])
            nc.scalar.copy(xT16[:, :, t * P:(t + 1) * P],
                           px.rearrange("p (j c) -> p j c", c=P))

        def stream_w_expert(e):
            # Stream W for one expert in 1 MiB chunks, each partition reading
            # 2 adjacent rows (contiguous bytes -> best HBM bandwidth), cast
            # to fp16.  This interleaves the contraction order: k-tile
            # (q, s) of expert e covers i-values {256q + 2p + s}; the x^T
            # tiles are built with the matching stride-2 column slices.
            for q in range(KI // 2):
                k0 = e * KI + q * 2
                r0 = e * IN_DIM + q * 2 * P
                if e == 0 and q == 0:
                    # Split the very first chunk so the first chain matmul
                    # can issue one DMA earlier.
                    src = Wf[r0:r0 + 2 * P, :].rearrange("(p s) o -> p s o",
                                                         s=2)
                    for s in range(2):
                        ws = stage.tile([P, 1, IN_DIM], f32, tag="wstg",
                                        bufs=2, name=f"ws0_{s}")
                        nc.sync.dma_start(ws[:], src[:, s:s + 1, :])
                        nc.vector.tensor_copy(W16[:, k0 + s:k0 + s + 1, :],
                                              ws[:])
                    continue
                ws = stage.tile([P, 2, IN_DIM], f32, tag="wstg", bufs=2,
                                name=f"ws_{e}_{q}")
                nc.sync.dma_start(
                    ws[:], Wf[r0:r0 + 2 * P, :].rearrange(
                        "(p s) o -> p s o", s=2))
                nc.vector.tensor_copy(W16[:, k0:k0 + 2, :], ws[:])

        stream_w_expert(0)
        for t in blocks[0]:
            prep_x_tile(t)

        # Bias w^T transposes, all upfront — the PE has slack while W streams.
        for t in range(TT):
            pw = psp.tile([P, P], f32, tag="ps", name=f"pw_{t}")
            nc.tensor.transpose(pw[:EXPERTS, :], w_sb[:, t, :], ident[:])
            nc.scalar.copy(wT16[:, t * P:(t + 1) * P], pw[:EXPERTS, :])

        # Remaining x tiles are prepped inside block 0's expert loop, where
        # the PE is intermittently DMA-starved anyway.
        prep_pending = list(range(BLK0, TT))

        for bi, btiles in enumerate(blocks):
            y0s = {}
            for t in btiles:
                y0s[t] = yaccp.tile([P, OUT_DIM], f32, tag="y0",
                                    name=f"y0_{t}")

            for e in range(EXPERTS):
                if bi == 0 and e + 1 < EXPERTS:
                    stream_w_expert(e + 1)
                for t in btiles:
                    tok = slice(t * P, (t + 1) * P)
                    y0 = y0s[t]
                    if e == 0:
                        # Bias init: y0 = w[t-tile, :] @ b (K=8 matmul).
                        pb0 = psp.tile([P, OC], f32, tag="ps")
                        pb1 = psp.tile([P, OC], f32, tag="ps")
                        nc.tensor.matmul(pb0[:], wT16[:, tok], b16[:, 0:OC],
                                         start=True, stop=True)
                        nc.tensor.matmul(pb1[:], wT16[:, tok], b16[:, OC:],
                                         start=True, stop=True)
                        nc.scalar.copy(y0[:, 0:OC], pb0[:])
                        nc.scalar.copy(y0[:, OC:], pb1[:])
                    ps0 = psp.tile([P, OC], f32, tag="ps")
                    ps1 = psp.tile([P, OC], f32, tag="ps")
                    for i in range(KI):
                        nc.tensor.matmul(ps0[:], xT16[:, i, tok],
                                         W16[:, e * KI + i, 0:OC],
                                         start=(i == 0), stop=(i == KI - 1))
                    for i in range(KI):
                        nc.tensor.matmul(ps1[:], xT16[:, i, tok],
                                         W16[:, e * KI + i, OC:],
                                         start=(i == 0), stop=(i == KI - 1))
                    wsc = w_sb[:, t, e:e + 1]
                    nc.vector.scalar_tensor_tensor(
                        y0[:, 0:OC], ps0[:], wsc, y0[:, 0:OC],
                        mybir.AluOpType.mult, mybir.AluOpType.add)
                    nc.vector.scalar_tensor_tensor(
                        y0[:, OC:], ps1[:], wsc, y0[:, OC:],
                        mybir.AluOpType.mult, mybir.AluOpType.add)
                    # Interleave the remaining x-tile preps into block 0.
                    if bi == 0 and e >= 1 and prep_pending:
                        if (e * len(btiles) + btiles.index(t)) % 2 == 0:
                            prep_x_tile(prep_pending.pop(0))

            for t in btiles:
                nc.sync.dma_start(y[t * P:(t + 1) * P, :], y0s[t][:])

        # Any preps not emitted inside block 0 (small-T configs).
        assert not prep_pending or TT <= BLK0, prep_pending


_NC_CACHE = None


def _build_nc(T=T, num_devices=N_CORES):
    global _NC_CACHE
    if T == BATCH // N_CORES and _NC_CACHE is not None:
        return _NC_CACHE
    nc = bacc.Bacc("TRN2", target_bir_lowering=False, debug=False,
                   num_devices=num_devices)
    x = nc.dram_tensor("x", [T, IN_DIM], f32, kind="ExternalInput").ap()
    w = nc.dram_tensor("weights", [T, EXPERTS], f32, kind="ExternalInput").ap()
    Wf = nc.dram_tensor("W", [EXPERTS * IN_DIM, OUT_DIM], f32,
                        kind="ExternalInput").ap()
    bf = nc.dram_tensor("b", [EXPERTS, OUT_DIM], f32, kind="ExternalInput").ap()
    y = nc.dram_tensor("y", [T, OUT_DIM], f32, kind="ExternalOutput").ap()
    with tile.TileContext(nc) as tc:
        _emit(tc, y, x, w, Wf, bf, T=T)
    nc.compile()
    if T == BATCH // N_CORES:
        _NC_CACHE = nc
    return nc


def _run(inputs, trace=False):
    nc = _build_nc()
    x = np.ascontiguousarray(np.asarray(inputs["x"], dtype=np.float32))
    w = np.ascontiguousarray(np.asarray(inputs["weights"], dtype=np.float32))
    W = np.ascontiguousarray(
        np.asarray(inputs["W"], dtype=np.float32).reshape(EXPERTS * IN_DIM,
                                                          OUT_DIM))
    b = np.ascontiguousarray(
        np.asarray(inputs["b"], dtype=np.float32).reshape(EXPERTS, OUT_DIM))
    in_maps = [
        {
            "x": x[c * T:(c + 1) * T],
            "weights": w[c * T:(c + 1) * T],
            "W": W,
            "b": b,
        }
        for c in range(N_CORES)
    ]
    try:
        res = run_bass_kernel_spmd(nc, in_maps, list(range(N_CORES)),
                                   trace=trace)
    except Exception:
        # One retry: the NRT exec unit occasionally reports a transient
        # unrecoverable error under this axon tunnel.
        res = run_bass_kernel_spmd(nc, in_maps, list(range(N_CORES)),
                                   trace=trace)
    y = np.concatenate([res.results[i]["y"] for i in range(N_CORES)], axis=0)
    return y, res


def kernel(x, weights, W, b):
    y, _ = _run({"x": x, "weights": weights, "W": W, "b": b})
    return y



# revision 4
# speedup vs baseline: 1.2342x; 1.2342x over previous
"""Trainium2 Bass kernel for nn_ExpertLinear (dense MoE routing).

y[t, o] = sum_e weights[t, e] * (x[t, :] @ W[e] + b[e])

Strategy
--------
Data-parallel over the batch across 8 NeuronCores (2048 tokens per core);
W and b are replicated.  Per core, a mean-split mixed-precision scheme:

    w[t, e] = wbar[t] + delta[t, e],   wbar = mean_e w[t, e]

  * Mean term  wbar[t] * (x[t] @ S),  S = sum_e W[e]:  ONE fp16 GEMM
    (1/8 of the FLOPs) carrying ~85% of the signal energy at fp16
    accuracy.  S is accumulated in fp16 on the DVE while W streams in.
  * Delta term sum_e delta[t,e] * (x[t] @ W[e]): all 8 GEMMs in fp8
    (e4m3) using the PE's DoubleRow perf mode -- each instruction
    contracts K=256 (two 128-k-tiles, 2 MACs/cell/cycle), i.e. 2x the
    fp16 matmul rate.  The fp8 quantization noise is scaled by
    |delta|/|w| ~ 0.47, keeping total rel err ~1.25e-2 (< 2e-2 gate).
    W8 holds 256*W so the tiny W values stay in e4m3's normal range;
    the 1/256 is folded into the delta scalars.
  * The routing scalars are applied output-side with a fused DVE
    scalar_tensor_tensor (y0 += d[:, e] * psum) using the per-partition
    scalar broadcast; y0 accumulates in fp16 (adds ~7e-4 rounding,
    negligible vs the fp8 noise).
  * The bias term sum_e w[t,e] b[e,o] is a K=8 fp16 matmul that
    initializes the accumulator (as in the fp16 baseline).
  * Pipeline: expert-outer / token-tile-inner.  Expert e+1's W streams
    (sync DMA queue) behind expert e's 16 tile-chains; x tiles load on
    the scalar queue, are transposed on the PE in fp16 and cast to a
    resident fp8 x^T.  The S-chains + bias + drain run per-tile after
    the last expert.
"""

import numpy as np

import concourse.bacc as bacc
import concourse.bass as bass
import concourse.mybir as mybir
import concourse.tile as tile
from concourse.bass_utils import run_bass_kernel_spmd
from concourse.masks import make_identity

EXPERTS = 8
IN_DIM = 1024
OUT_DIM = 1024
BATCH = 16384
N_CORES = 8

P = 128                 # partitions
T = BATCH // N_CORES    # tokens per core (2048)
TT = T // P             # token tiles per core (16)
KI = IN_DIM // P        # contraction tiles per expert (8)
NK = EXPERTS * KI       # total contraction tiles (64)
OC = 512                # psum free-dim chunk (one fp32 PSUM bank)

W8_SCALE = 256.0        # W is ~U(-0.0024, 0.0024): scale into e4m3 range

f32 = mybir.dt.float32
f16 = mybir.dt.float16
f8 = mybir.dt.float8e4
DR = mybir.MatmulPerfMode.DoubleRow
ALU = mybir.AluOpType
AX = mybir.AxisListType


def _emit(tc, y, x, w, Wf, bf, T=T):
    nc = tc.nc
    TT = T // P

    with (
        tc.tile_pool(name="big", bufs=1) as big,
        tc.tile_pool(name="stage", bufs=2) as stage,
        tc.tile_pool(name="ps", bufs=8, space="PSUM") as psp,
    ):
        ident = big.tile([P, P], f32)
        make_identity(nc, ident)
        ident16 = big.tile([P, P], f16)
        nc.vector.tensor_copy(ident16[:], ident[:])

        # Routing weights, token-on-partition layout: w_sb[p, t, e].
        w_sb = big.tile([P, TT, EXPERTS], f32)
        for t in range(TT):
            nc.gpsimd.dma_start(w_sb[:, t, :], w[t * P:(t + 1) * P, :])

        # Bias in fp16, experts on partitions (casting DMA on SWDGE).
        b16 = big.tile([EXPERTS, OUT_DIM], f16)
        nc.gpsimd.dma_start(b16[:], bf[:])

        # wbar = mean_e w, ds = (w - wbar)/W8_SCALE (delta-apply scalars).
        wbar = big.tile([P, TT], f32)
        nc.vector.tensor_reduce(wbar[:], w_sb[:], AX.X, ALU.add)
        nc.vector.tensor_scalar(wbar[:], wbar[:], 1.0 / EXPERTS, None,
                                ALU.mult)
        ds = big.tile([P, TT, EXPERTS], f32)
        for t in range(TT):
            nc.vector.tensor_scalar(ds[:, t, :], w_sb[:, t, :],
                                    wbar[:, t:t + 1], 1.0 / W8_SCALE,
                                    ALU.subtract, ALU.mult)

        # w^T tiles for the bias matmuls.
        wT16 = big.tile([EXPERTS, TT * P], f16)

        W8 = big.tile([P, NK, OUT_DIM], f8)     # 256*W [(e,i), o], resident
        S16 = big.tile([P, KI, OUT_DIM], f16)   # sum_e W[e], fp16
        nc.vector.memset(S16[:], 0.0)
        xT16 = big.tile([P, KI, T], f16)        # x^T [i, tok], fp16 resident
        xT8 = big.tile([P, KI, T], f8)          # x^T in fp8
        y0s = [big.tile([P, OUT_DIM], f16, name=f"y0_{t}")
               for t in range(TT)]

        def prep_x_tile(t):
            """Load one x tile (scalar HWDGE queue), cast to fp16, PE-
            transpose into resident x^T, cast the slice to fp8."""
            xs = stage.tile([P, IN_DIM], f32, tag="xstg", bufs=3,
                            name=f"xs_{t}")
            nc.scalar.dma_start(xs[:], x[t * P:(t + 1) * P, :])
            x16s = stage.tile([P, IN_DIM], f16, tag="x16s", bufs=2,
                              name=f"x16s_{t}")
            nc.vector.tensor_copy(x16s[:], xs[:])
            px = psp.tile([P, KI * P], f16, tag="ps", name=f"px_{t}")
            for j in range(KI):
                q, s = divmod(j, 2)
                nc.tensor.transpose(px[:, j * P:(j + 1) * P],
                                    x16s[:, 2 * P * q + s::2][:, :P],
                                    ident16[:])
            tok = slice(t * P, (t + 1) * P)
            nc.scalar.copy(xT16[:, :, tok],
                           px.rearrange("p (j c) -> p j c", c=P))
            nc.vector.tensor_copy(xT8[:, :, tok], xT16[:, :, tok])

        def stream_w_expert(e):
            # Stream W for one expert in 1 MiB chunks, each partition
            # reading 2 adjacent rows (contiguous bytes); k-tile (q, s) of
            # expert e covers i-values {256q + 2p + s}, matching the x^T
            # stride-2 column slices.  Each chunk is cast *256 into the
            # resident fp8 W8 and accumulated into S16.
            for q in range(KI // 2):
                k0 = e * KI + q * 2
                r0 = e * IN_DIM + q * 2 * P
                ws = stage.tile([P, 2, IN_DIM], f32, tag="wstg", bufs=2,
                                name=f"ws_{e}_{q}")
                nc.sync.dma_start(
                    ws[:], Wf[r0:r0 + 2 * P, :].rearrange(
                        "(p s) o -> p s o", s=2))
                nc.vector.tensor_scalar(W8[:, k0:k0 + 2, :], ws[:],
                                        W8_SCALE, None, ALU.mult)
                nc.vector.tensor_tensor(S16[:, 2 * q:2 * q + 2, :],
                                        S16[:, 2 * q:2 * q + 2, :], ws[:],
                                        ALU.add)

        stream_w_expert(0)
        for t in range(min(3, TT)):
            prep_x_tile(t)

        # Bias w^T transposes, all upfront -- the PE has slack while W
        # streams.
        for t in range(TT):
            pw = psp.tile([P, P], f32, tag="ps", name=f"pw_{t}")
            nc.tensor.transpose(pw[:EXPERTS, :], w_sb[:, t, :], ident[:])
            nc.scalar.copy(wT16[:, t * P:(t + 1) * P], pw[:EXPERTS, :])

        prep_pending = list(range(3, TT))

        for e in range(EXPERTS):
            if e + 1 < EXPERTS:
                stream_w_expert(e + 1)
            for t in range(TT):
                tok = slice(t * P, (t + 1) * P)
                y0 = y0s[t]
                if e == 0:
                    if prep_pending:
                        prep_x_tile(prep_pending.pop(0))
                    # Bias init: y0 = w[t-tile, :] @ b (K=8 matmul).
                    pb0 = psp.tile([P, OC], f32, tag="ps")
                    pb1 = psp.tile([P, OC], f32, tag="ps")
                    nc.tensor.matmul(pb0[:], wT16[:, tok], b16[:, 0:OC],
                                     start=True, stop=True)
                    nc.tensor.matmul(pb1[:], wT16[:, tok], b16[:, OC:],
                                     start=True, stop=True)
                    nc.scalar.copy(y0[:, 0:OC], pb0[:])
                    nc.scalar.copy(y0[:, OC:], pb1[:])
                # Delta chains: 4 DoubleRow matmuls per 512-out half,
                # each contracting K=256 (two k-tiles).
                ps0 = psp.tile([P, OC], f32, tag="ps")
                ps1 = psp.tile([P, OC], f32, tag="ps")
                for qq in range(KI // 2):
                    nc.tensor.matmul(ps0[:],
                                     xT8[:, 2 * qq:2 * qq + 2, tok],
                                     W8[:, e * KI + 2 * qq:
                                        e * KI + 2 * qq + 2, 0:OC],
                                     start=(qq == 0), stop=(qq == 3),
                                     perf_mode=DR)
                for qq in range(KI // 2):
                    nc.tensor.matmul(ps1[:],
                                     xT8[:, 2 * qq:2 * qq + 2, tok],
                                     W8[:, e * KI + 2 * qq:
                                        e * KI + 2 * qq + 2, OC:],
                                     start=(qq == 0), stop=(qq == 3),
                                     perf_mode=DR)
                dsc = ds[:, t, e:e + 1]
                nc.vector.scalar_tensor_tensor(
                    y0[:, 0:OC], ps0[:], dsc, y0[:, 0:OC],
                    ALU.mult, ALU.add)
                nc.vector.scalar_tensor_tensor(
                    y0[:, OC:], ps1[:], dsc, y0[:, OC:],
                    ALU.mult, ALU.add)
                if e == EXPERTS - 1:
                    # Mean term: y0 += wbar[t] * (x @ S16), fp16 chain.
                    psS0 = psp.tile([P, OC], f32, tag="ps")
                    psS1 = psp.tile([P, OC], f32, tag="ps")
                    for i in range(KI):
                        nc.tensor.matmul(psS0[:], xT16[:, i, tok],
                                         S16[:, i, 0:OC],
                                         start=(i == 0), stop=(i == KI - 1))
                    for i in range(KI):
                        nc.tensor.matmul(psS1[:], xT16[:, i, tok],
                                         S16[:, i, OC:],
                                         start=(i == 0), stop=(i == KI - 1))
                    wsc = wbar[:, t:t + 1]
                    nc.vector.scalar_tensor_tensor(
                        y0[:, 0:OC], psS0[:], wsc, y0[:, 0:OC],
                        ALU.mult, ALU.add)
                    nc.vector.scalar_tensor_tensor(
                        y0[:, OC:], psS1[:], wsc, y0[:, OC:],
                        ALU.mult, ALU.add)
                    nc.gpsimd.dma_start(y[tok, :], y0[:])


_NC_CACHE = None


def _build_nc(T=T, num_devices=N_CORES):
    global _NC_CACHE
    if T == BATCH // N_CORES and _NC_CACHE is not None:
        return _NC_CACHE
    nc = bacc.Bacc("TRN2", target_bir_lowering=False, debug=False,
                   num_devices=num_devices)
    x = nc.dram_tensor("x", [T, IN_DIM], f32, kind="ExternalInput").ap()
    w = nc.dram_tensor("weights", [T, EXPERTS], f32, kind="ExternalInput").ap()
    Wf = nc.dram_tensor("W", [EXPERTS * IN_DIM, OUT_DIM], f32,
                        kind="ExternalInput").ap()
    bf = nc.dram_tensor("b", [EXPERTS, OUT_DIM], f32, kind="ExternalInput").ap()
    y = nc.dram_tensor("y", [T, OUT_DIM], f32, kind="ExternalOutput").ap()
    with tile.TileContext(nc) as tc:
        _emit(tc, y, x, w, Wf, bf, T=T)
    nc.compile()
    if T == BATCH // N_CORES:
        _NC_CACHE = nc
    return nc


def _run(inputs, trace=False):
    nc = _build_nc()
    x = np.ascontiguousarray(np.asarray(inputs["x"], dtype=np.float32))
    w = np.ascontiguousarray(np.asarray(inputs["weights"], dtype=np.float32))
    W = np.ascontiguousarray(
        np.asarray(inputs["W"], dtype=np.float32).reshape(EXPERTS * IN_DIM,
                                                          OUT_DIM))
    b = np.ascontiguousarray(
        np.asarray(inputs["b"], dtype=np.float32).reshape(EXPERTS, OUT_DIM))
    in_maps = [
        {
            "x": x[c * T:(c + 1) * T],
            "weights": w[c * T:(c + 1) * T],
            "W": W,
            "b": b,
        }
        for c in range(N_CORES)
    ]
    try:
        res = run_bass_kernel_spmd(nc, in_maps, list(range(N_CORES)),
                                   trace=trace)
    except Exception:
        # One retry: the NRT exec unit occasionally reports a transient
        # unrecoverable error under this axon tunnel.
        res = run_bass_kernel_spmd(nc, in_maps, list(range(N_CORES)),
                                   trace=trace)
    y = np.concatenate([res.results[i]["y"] for i in range(N_CORES)], axis=0)
    return y, res


def kernel(x, weights, W, b):
    y, _ = _run({"x": x, "weights": weights, "W": W, "b": b})
    return y
